# revision 36
# baseline (speedup 1.0000x reference)
"""Multi-head self-attention (B=2, S=2048, D=1024, H=16) on 8 TRN2 NeuronCores.

Tensor-parallel over heads: each core owns 2 heads. Accepts FULL inputs,
returns FULL output. Host pre-transposes x and slices per-head weights;
each core computes qkv -> per-head LayerNorm -> attention -> partial
output projection (over its 128 embed dims); host sums the 8 partials.
"""

import os
import sys

import numpy as np

for _p in ("/opt/trn_rl_repo", "/root/.axon_site/_ro/trn_rl_repo"):
    if os.path.isdir(_p) and _p not in sys.path:
        sys.path.insert(0, _p)
        break

import concourse.bass as bass  # noqa: E402
import concourse.bacc as bacc  # noqa: E402
import concourse.tile as tile  # noqa: E402
from concourse import mybir  # noqa: E402
from concourse.bass_utils import run_bass_kernel_spmd  # noqa: E402

F32 = mybir.dt.float32
F32R = mybir.dt.float32r
BF16 = mybir.dt.bfloat16
AF = mybir.ActivationFunctionType

NCORES = 8
D = 1024
H = 16
HD = 64
HPC = H // NCORES          # heads per core = 2
DPC = HPC * HD             # embed dims per core = 128
EPS = 1e-5


def _r(ap):
    return ap.bitcast(F32R)


def build_nc(B, S, affine):
    """Build the SPMD Bass program for one core (same program, 8 cores)."""
    T = B * S                      # total token columns
    NTB = T // 128                 # 128-token blocks
    NCH = T // 512                 # 512-token chunks
    QC = S // 512                  # q-chunks per batch
    KB = S // 128                  # k-blocks per batch
    KCH = D // 128                 # contraction chunks (8)
    SCALE = 1.0 / np.sqrt(HD)

    nc = bacc.Bacc(
        "TRN2",
        target_bir_lowering=False,
        debug=False,
        enable_asserts=True,
        num_devices=NCORES,
    )

    xT = nc.dram_tensor("xT", [D, T], BF16, kind="ExternalInput").ap()
    wq = nc.dram_tensor("wt_qkv", [D, 3 * DPC], BF16, kind="ExternalInput").ap()
    bq = nc.dram_tensor("b_qkv_s", [1, 3 * DPC], BF16, kind="ExternalInput").ap()
    wp = nc.dram_tensor("wt_proj", [DPC, D], BF16, kind="ExternalInput").ap()
    bp = nc.dram_tensor("b_proj_s", [1, D], BF16, kind="ExternalInput").ap()
    ones = nc.dram_tensor("c_ones", [1, 512], F32R, kind="ExternalInput").ap()
    vones = nc.dram_tensor(
        "c_vones", [128, HPC, NTB, 1], BF16, kind="ExternalInput"
    ).ap()
    onesb = nc.dram_tensor("c_onesb", [1, 512], BF16, kind="ExternalInput").ap()
    eye = nc.dram_tensor("c_eye", [128, 128], BF16, kind="ExternalInput").ap()
    if affine:
        gb = nc.dram_tensor("c_gb", [128, 4, HD], F32, kind="ExternalInput").ap()
    outp = nc.dram_tensor("outp", [T, D], F32, kind="ExternalOutput").ap()

    from contextlib import ExitStack

    with tile.TileContext(nc) as tc, ExitStack() as stack:
        const = stack.enter_context(tc.tile_pool(name="const", bufs=1))
        persist = stack.enter_context(tc.tile_pool(name="persist", bufs=1))

        wq_sb = const.tile([128, KCH, 3 * DPC], BF16, tag="wq")
        nc.sync.dma_start(
            out=wq_sb, in_=wq.rearrange("(c p) n -> p c n", p=128)
        )
        wp_sb = const.tile([DPC, D], BF16, tag="wp")
        nc.sync.dma_start(out=wp_sb, in_=wp)
        bq_sb = const.tile([1, 3 * DPC], BF16, tag="bq")
        nc.sync.dma_start(out=bq_sb, in_=bq)
        bp_sb = const.tile([1, D], BF16, tag="bp")
        nc.sync.dma_start(out=bp_sb, in_=bp)
        ones_sb = const.tile([1, 512], F32R, tag="ones")
        nc.sync.dma_start(out=ones_sb, in_=ones)
        onesb_sb = const.tile([1, 512], BF16, tag="onesb")
        nc.sync.dma_start(out=onesb_sb, in_=onesb)
        eye_sb = const.tile([128, 128], BF16, tag="eye")
        nc.sync.dma_start(out=eye_sb, in_=eye)
        eps_sb = const.tile([128, 1], F32, tag="eps")
        nc.vector.memset(eps_sb, EPS)

        if affine:
            gb_sb = const.tile([128, 4, HD], F32, tag="gb")
            nc.sync.dma_start(out=gb_sb, in_=gb)

        # persistent intermediates
        qT = persist.tile([128, T], BF16, tag="qT")     # [2h*64, tok] LN'd q^T
        kT = persist.tile([128, T], BF16, tag="kT")
        vO = persist.tile([128, HPC, NTB, HD + 1], BF16, tag="vO")  # v + ones col
        aT = persist.tile([128, T], BF16, tag="aT")     # attention out^T
        nc.sync.dma_start(out=vO[:, :, :, HD : HD + 1], in_=vones)

        # ---------------- Phase 1: qkv + LayerNorm + transpose ----------
        with (
            tc.tile_pool(name="xt", bufs=2) as xt_pool,
            tc.tile_pool(name="qkv_ps", bufs=2, space="PSUM") as qkv_ps,
            tc.tile_pool(name="t_ps", bufs=2, space="PSUM") as t_ps,
            tc.tile_pool(name="stage1", bufs=3) as stage1,
            tc.tile_pool(name="stats", bufs=3) as stats_pool,
        ):
            for n in range(NCH):
                xt = xt_pool.tile([128, KCH, 512], BF16, tag="xt")
                nc.sync.dma_start(
                    out=xt,
                    in_=xT.rearrange("(c p) t -> p c t", p=128)[
                        :, :, n * 512 : (n + 1) * 512
                    ],
                )
                for tbl in range(4):
                    tb = n * 4 + tbl
                    ps = qkv_ps.tile([128, 3 * DPC], F32, tag="ps")
                    nc.tensor.matmul(
                        ps,
                        lhsT=onesb_sb[0:1, 0:128],
                        rhs=bq_sb,
                        start=True,
                        stop=False,
                    )
                    for k in range(KCH):
                        nc.tensor.matmul(
                            ps,
                            lhsT=xt[:, k, tbl * 128 : (tbl + 1) * 128],
                            rhs=wq_sb[:, k, :],
                            start=False,
                            stop=(k == KCH - 1),
                        )
                    # LayerNorm over each head's 64 dims of q and k
                    qk = ps[:, 0 : 2 * DPC].rearrange("p (g d) -> p g d", d=HD)
                    st = stats_pool.tile([128, 4, 6], F32, tag="st")
                    mv = stats_pool.tile([128, 4, 2], F32, tag="mv")
                    for g in range(4):
                        nc.vector.bn_stats(out=st[:, g, :], in_=qk[:, g, :])
                        nc.vector.bn_aggr(out=mv[:, g, :], in_=st[:, g, :])
                    rstd = stats_pool.tile([128, 4], F32, tag="rstd")
                    nc.scalar.activation(
                        out=rstd, in_=mv[:, :, 1], func=AF.Sqrt, bias=eps_sb
                    )
                    nc.vector.reciprocal(out=rstd, in_=rstd)
                    qn = stage1.tile([128, 128], BF16, tag="qn")
                    kn = stage1.tile([128, 128], BF16, tag="kn")
                    for g in range(4):
                        dst = qn if g < 2 else kn
                        dsl = dst[:, (g % 2) * HD : (g % 2 + 1) * HD]
                        nc.vector.tensor_scalar(
                            out=dsl,
                            in0=qk[:, g, :],
                            scalar1=mv[:, g, 0:1],
                            scalar2=rstd[:, g : g + 1],
                            op0=mybir.AluOpType.subtract,
                            op1=mybir.AluOpType.mult,
                        )
                        if affine:
                            nc.vector.tensor_mul(dsl, dsl, gb_sb[:, 2 * (g // 2), :])
                            nc.vector.tensor_add(
                                dsl, dsl, gb_sb[:, 2 * (g // 2) + 1, :]
                            )
                    # v (+ ones col already set)
                    for h in range(HPC):
                        nc.vector.tensor_copy(
                            out=vO[:, h, tb, 0:HD],
                            in_=ps[:, 2 * DPC + h * HD : 2 * DPC + (h + 1) * HD],
                        )
                    # transpose q,k into [dim, token] layout
                    tp = t_ps.tile([128, 256], BF16, tag="tp")
                    nc.tensor.transpose(tp[:, 0:128], qn, eye_sb)
                    nc.tensor.transpose(tp[:, 128:256], kn, eye_sb)
                    ts = slice(tb * 128, (tb + 1) * 128)
                    nc.vector.tensor_copy(out=qT[:, ts], in_=tp[:, 0:128])
                    nc.scalar.copy(out=kT[:, ts], in_=tp[:, 128:256])

        # ---------------- Phase 2: attention ----------------------------
        with (
            tc.tile_pool(name="sc_ps", bufs=2, space="PSUM") as sc_ps,
            tc.tile_pool(name="o_ps", bufs=1, space="PSUM") as o_ps,
            tc.tile_pool(name="rb_ps", bufs=1, space="PSUM") as rb_ps,
            tc.tile_pool(name="exps", bufs=3) as exps,
            tc.tile_pool(name="stage2", bufs=3) as stage2,
            tc.tile_pool(name="pj_ps", bufs=1, space="PSUM") as pj_ps,
            tc.tile_pool(name="ostage", bufs=4) as ostage,
        ):
            for b in range(B):
                for qc in range(QC):
                    cols = slice(b * S + qc * 512, b * S + (qc + 1) * 512)
                    oo = [
                        o_ps.tile([HD + 1, 512], F32, tag=f"o{h}", name=f"o{h}")
                        for h in range(HPC)
                    ]
                    for kb in range(KB):
                        gkb = b * KB + kb
                        ks = slice(gkb * 128, (gkb + 1) * 128)
                        # issue the two heads' score matmuls back-to-back:
                        # K=64 each at partition bases 0/64 -> disjoint row
                        # groups, so the PE runs them concurrently
                        # both heads' scores into one 2-bank PSUM tile so
                        # a single 1024-wide exp covers them (halves the ACT
                        # per-op overhead and the PE's exp-wait points)
                        scp = sc_ps.tile(
                            [128, HPC, 512], F32, tag="s", name="scp"
                        )
                        for h in range(HPC):
                            hp = slice(h * HD, (h + 1) * HD)
                            nc.tensor.matmul(
                                scp[:, h, :],
                                lhsT=kT[hp, ks],
                                rhs=qT[hp, cols],
                                start=True,
                                stop=True,
                            )
                        ex = exps.tile(
                            [128, HPC, 512], BF16, tag="ex", name="ex"
                        )
                        nc.scalar.activation(
                            out=ex, in_=scp, func=AF.Exp, scale=SCALE
                        )
                        for h in range(HPC):
                            nc.tensor.matmul(
                                oo[h],
                                lhsT=vO[:, h, gkb, :],
                                rhs=ex[:, h, :],
                                start=(kb == 0),
                                stop=(kb == KB - 1),
                            )
                    for h in range(HPC):
                        # 1/denom broadcast along 64 partitions without a
                        # (slow, single-lane) DVE reciprocal: ln(d) on the
                        # denom row, outer-product broadcast on the PE, then
                        # exp(-x) on ScalarE (same ACT table set as softmax).
                        rc = stage2.tile([1, 512], F32R, tag=f"rc{h}")
                        nc.scalar.activation(
                            out=rc, in_=oo[h][HD : HD + 1, :], func=AF.Ln
                        )
                        rb = rb_ps.tile([HD, 512], F32, tag="rb", name="rb")
                        nc.tensor.matmul(
                            rb,
                            lhsT=ones_sb[0:1, 0:HD],
                            rhs=rc,
                            start=True,
                            stop=True,
                        )
                        rbs = stage2.tile([HD, 512], F32, tag=f"rbs{h}")
                        nc.scalar.activation(
                            out=rbs, in_=rb, func=AF.Exp, scale=-1.0
                        )
                        nc.vector.tensor_mul(
                            aT[h * HD : (h + 1) * HD, cols], oo[h][0:HD, :], rbs
                        )
                    # fused partial projection for the 4 token blocks of
                    # this q-chunk (keeps the PE fed while the next chunk's
                    # exp tiles are produced; removes the separate proj tail)
                    for tbl in range(4):
                        tb = (b * QC + qc) * 4 + tbl
                        ob = ostage.tile([128, D], F32, tag="ob")
                        for nn in range(D // 512):
                            pps = pj_ps.tile(
                                [128, 512], F32, tag="pj", name="pps"
                            )
                            nc.tensor.matmul(
                                pps,
                                lhsT=onesb_sb[0:1, 0:128],
                                rhs=bp_sb[0:1, nn * 512 : (nn + 1) * 512],
                                start=True,
                                stop=False,
                            )
                            nc.tensor.matmul(
                                pps,
                                lhsT=aT[:, tb * 128 : (tb + 1) * 128],
                                rhs=wp_sb[:, nn * 512 : (nn + 1) * 512],
                                start=False,
                                stop=True,
                            )
                            if nn == 0:
                                nc.scalar.copy(
                                    out=ob[:, nn * 512 : (nn + 1) * 512],
                                    in_=pps,
                                )
                            else:
                                nc.vector.tensor_copy(
                                    out=ob[:, nn * 512 : (nn + 1) * 512],
                                    in_=pps,
                                )
                        nc.sync.dma_start(
                            out=outp[tb * 128 : (tb + 1) * 128, :], in_=ob
                        )

    nc.compile()
    return nc


def make_in_maps(x, w_qkv, b_qkv, w_proj, b_proj, q_gamma, q_beta, k_gamma, k_beta,
                 affine):
    B, S, _ = x.shape
    T = B * S
    xT = np.ascontiguousarray(x.reshape(T, D).T)
    ones = np.ones((1, 512), np.float32)
    eye = np.eye(128, dtype=np.float32)
    in_maps = []
    for c in range(NCORES):
        rs = slice(c * DPC, (c + 1) * DPC)
        w_slice = np.concatenate(
            [w_qkv[rs], w_qkv[D:2 * D][rs.start:rs.stop], w_qkv[2 * D:][rs.start:rs.stop]],
            axis=0,
        )  # [384, 1024]
        b_slice = np.concatenate(
            [b_qkv[rs], b_qkv[D:2 * D][rs.start:rs.stop], b_qkv[2 * D:][rs.start:rs.stop]]
        )[None, :]  # [1, 384]
        import ml_dtypes
        bf = ml_dtypes.bfloat16
        m = {
            "xT": xT.astype(bf),
            "wt_qkv": np.ascontiguousarray(w_slice.T).astype(bf),
            "b_qkv_s": np.ascontiguousarray(b_slice).astype(bf),
            "wt_proj": np.ascontiguousarray(w_proj[:, rs].T).astype(bf),
            "b_proj_s": (b_proj[None, :] if c == 0
                         else np.zeros((1, D), np.float32)).astype(bf),
            "c_ones": ones,
            "c_vones": np.ones((128, HPC, (T // 128), 1), bf),
            "c_onesb": np.ones((1, 512), bf),
            "c_eye": eye.astype(bf),
        }
        if affine:
            gb = np.stack([q_gamma, q_beta, k_gamma, k_beta])  # [4, 64]
            m["c_gb"] = np.ascontiguousarray(
                np.broadcast_to(gb[None], (128, 4, HD)).astype(np.float32)
            )
        in_maps.append(m)
    return in_maps


_NC_CACHE = {}

LAST_RESULTS = None


def kernel(x, w_qkv, b_qkv, w_proj, b_proj, q_gamma, q_beta, k_gamma, k_beta,
           **unused):
    global LAST_RESULTS
    x = np.asarray(x, np.float32)
    w_qkv = np.asarray(w_qkv, np.float32)
    b_qkv = np.asarray(b_qkv, np.float32)
    w_proj = np.asarray(w_proj, np.float32)
    b_proj = np.asarray(b_proj, np.float32)
    q_gamma = np.asarray(q_gamma, np.float32)
    q_beta = np.asarray(q_beta, np.float32)
    k_gamma = np.asarray(k_gamma, np.float32)
    k_beta = np.asarray(k_beta, np.float32)

    B, S, _ = x.shape
    affine = not (
        np.all(q_gamma == 1) and np.all(k_gamma == 1)
        and np.all(q_beta == 0) and np.all(k_beta == 0)
    )
    key = (B, S, affine)
    if key not in _NC_CACHE:
        _NC_CACHE[key] = build_nc(B, S, affine)
    nc = _NC_CACHE[key]

    in_maps = make_in_maps(
        x, w_qkv, b_qkv, w_proj, b_proj, q_gamma, q_beta, k_gamma, k_beta, affine
    )
    trace = bool(int(os.environ.get("BASS_KERNEL_TRACE", "0")))
    res = run_bass_kernel_spmd(
        nc, in_maps, core_ids=list(range(NCORES)), trace=trace
    )
    LAST_RESULTS = res
    acc = np.zeros((B * S, D), np.float32)
    for r in res.results:
        acc += r["outp"]
    return acc.reshape(B, S, D)


# revision 37
# speedup vs baseline: 1.0924x; 1.0924x over previous
"""Multi-head self-attention (B=2, S=2048, D=1024, H=16) on 8 TRN2 NeuronCores.

Tensor-parallel over heads: each core owns 2 heads. Accepts FULL inputs,
returns FULL output. Host pre-transposes x and slices per-head weights;
each core computes qkv -> per-head LayerNorm -> attention -> partial
output projection (over its 128 embed dims); host sums the 8 partials.
"""

import os
import sys

import numpy as np

for _p in ("/opt/trn_rl_repo", "/root/.axon_site/_ro/trn_rl_repo"):
    if os.path.isdir(_p) and _p not in sys.path:
        sys.path.insert(0, _p)
        break

import concourse.bass as bass  # noqa: E402
import concourse.bacc as bacc  # noqa: E402
import concourse.tile as tile  # noqa: E402
from concourse import mybir  # noqa: E402
from concourse.bass_utils import run_bass_kernel_spmd  # noqa: E402

F32 = mybir.dt.float32
F32R = mybir.dt.float32r
BF16 = mybir.dt.bfloat16
AF = mybir.ActivationFunctionType

NCORES = 8
D = 1024
H = 16
HD = 64
HPC = H // NCORES          # heads per core = 2
DPC = HPC * HD             # embed dims per core = 128
EPS = 1e-5


def _r(ap):
    return ap.bitcast(F32R)


def build_nc(B, S, affine):
    """Build the SPMD Bass program for one core (same program, 8 cores)."""
    T = B * S                      # total token columns
    NTB = T // 128                 # 128-token blocks
    NCH = T // 512                 # 512-token chunks
    QC = S // 512                  # q-chunks per batch
    KB = S // 128                  # k-blocks per batch
    KCH = D // 128                 # contraction chunks (8)
    SCALE = 1.0 / np.sqrt(HD)

    nc = bacc.Bacc(
        "TRN2",
        target_bir_lowering=False,
        debug=False,
        enable_asserts=True,
        num_devices=NCORES,
    )

    xT = nc.dram_tensor("xT", [D, T], BF16, kind="ExternalInput").ap()
    wq = nc.dram_tensor("wt_qkv", [D, 3 * DPC], BF16, kind="ExternalInput").ap()
    bq = nc.dram_tensor("b_qkv_s", [1, 3 * DPC], BF16, kind="ExternalInput").ap()
    wp = nc.dram_tensor("wt_proj", [DPC, D], BF16, kind="ExternalInput").ap()
    bp = nc.dram_tensor("b_proj_s", [1, D], BF16, kind="ExternalInput").ap()
    ones = nc.dram_tensor("c_ones", [1, 512], F32R, kind="ExternalInput").ap()
    vones = nc.dram_tensor(
        "c_vones", [128, HPC, NTB, 1], BF16, kind="ExternalInput"
    ).ap()
    onesb = nc.dram_tensor("c_onesb", [1, 512], BF16, kind="ExternalInput").ap()
    eye = nc.dram_tensor("c_eye", [128, 128], BF16, kind="ExternalInput").ap()
    if affine:
        gb = nc.dram_tensor("c_gb", [128, 4, HD], F32, kind="ExternalInput").ap()
    outp = nc.dram_tensor("outp", [T, D], F32, kind="ExternalOutput").ap()

    from contextlib import ExitStack

    with tile.TileContext(nc) as tc, ExitStack() as stack:
        const = stack.enter_context(tc.tile_pool(name="const", bufs=1))
        persist = stack.enter_context(tc.tile_pool(name="persist", bufs=1))

        wq_sb = const.tile([128, KCH, 3 * DPC], BF16, tag="wq")
        nc.sync.dma_start(
            out=wq_sb, in_=wq.rearrange("(c p) n -> p c n", p=128)
        )
        wp_sb = const.tile([DPC, D], BF16, tag="wp")
        nc.sync.dma_start(out=wp_sb, in_=wp)
        bq_sb = const.tile([1, 3 * DPC], BF16, tag="bq")
        nc.sync.dma_start(out=bq_sb, in_=bq)
        bp_sb = const.tile([1, D], BF16, tag="bp")
        nc.sync.dma_start(out=bp_sb, in_=bp)
        ones_sb = const.tile([1, 512], F32R, tag="ones")
        nc.sync.dma_start(out=ones_sb, in_=ones)
        onesb_sb = const.tile([1, 512], BF16, tag="onesb")
        nc.sync.dma_start(out=onesb_sb, in_=onesb)
        eye_sb = const.tile([128, 128], BF16, tag="eye")
        nc.sync.dma_start(out=eye_sb, in_=eye)
        eps_sb = const.tile([128, 1], F32, tag="eps")
        nc.vector.memset(eps_sb, EPS)

        if affine:
            gb_sb = const.tile([128, 4, HD], F32, tag="gb")
            nc.sync.dma_start(out=gb_sb, in_=gb)

        # persistent intermediates
        qT = persist.tile([128, T], BF16, tag="qT")     # [2h*64, tok] LN'd q^T
        kT = persist.tile([128, T], BF16, tag="kT")
        vO = persist.tile([128, HPC, NTB, HD + 1], BF16, tag="vO")  # v + ones col
        aT = persist.tile([128, T], BF16, tag="aT")     # attention out^T
        nc.sync.dma_start(out=vO[:, :, :, HD : HD + 1], in_=vones)

        # ---------------- Phase 1: qkv + LayerNorm + transpose ----------
        with (
            tc.tile_pool(name="xt", bufs=2) as xt_pool,
            tc.tile_pool(name="qkv_ps", bufs=3, space="PSUM") as qkv_ps,
            tc.tile_pool(name="t_ps", bufs=2, space="PSUM") as t_ps,
            tc.tile_pool(name="stage1", bufs=3) as stage1,
            tc.tile_pool(name="stats", bufs=3) as stats_pool,
        ):
            for n in range(NCH):
                xt = xt_pool.tile([128, KCH, 512], BF16, tag="xt")
                nc.sync.dma_start(
                    out=xt,
                    in_=xT.rearrange("(c p) t -> p c t", p=128)[
                        :, :, n * 512 : (n + 1) * 512
                    ],
                )
                for tbl in range(4):
                    tb = n * 4 + tbl
                    ps = qkv_ps.tile([128, 3 * DPC], F32, tag="ps")
                    nc.tensor.matmul(
                        ps,
                        lhsT=onesb_sb[0:1, 0:128],
                        rhs=bq_sb,
                        start=True,
                        stop=False,
                    )
                    for k in range(KCH):
                        nc.tensor.matmul(
                            ps,
                            lhsT=xt[:, k, tbl * 128 : (tbl + 1) * 128],
                            rhs=wq_sb[:, k, :],
                            start=False,
                            stop=(k == KCH - 1),
                        )
                    # LayerNorm over each head's 64 dims of q and k
                    qk = ps[:, 0 : 2 * DPC].rearrange("p (g d) -> p g d", d=HD)
                    st = stats_pool.tile([128, 4, 6], F32, tag="st")
                    mv = stats_pool.tile([128, 4, 2], F32, tag="mv")
                    for g in range(4):
                        nc.vector.bn_stats(out=st[:, g, :], in_=qk[:, g, :])
                        nc.vector.bn_aggr(out=mv[:, g, :], in_=st[:, g, :])
                    rstd = stats_pool.tile([128, 4], F32, tag="rstd")
                    nc.scalar.activation(
                        out=rstd, in_=mv[:, :, 1], func=AF.Sqrt, bias=eps_sb
                    )
                    nc.vector.reciprocal(out=rstd, in_=rstd)
                    qn = stage1.tile([128, 128], BF16, tag="qn")
                    kn = stage1.tile([128, 128], BF16, tag="kn")
                    for g in range(4):
                        dst = qn if g < 2 else kn
                        dsl = dst[:, (g % 2) * HD : (g % 2 + 1) * HD]
                        nc.vector.tensor_scalar(
                            out=dsl,
                            in0=qk[:, g, :],
                            scalar1=mv[:, g, 0:1],
                            scalar2=rstd[:, g : g + 1],
                            op0=mybir.AluOpType.subtract,
                            op1=mybir.AluOpType.mult,
                        )
                        if affine:
                            nc.vector.tensor_mul(dsl, dsl, gb_sb[:, 2 * (g // 2), :])
                            nc.vector.tensor_add(
                                dsl, dsl, gb_sb[:, 2 * (g // 2) + 1, :]
                            )
                    # v (+ ones col already set)
                    for h in range(HPC):
                        nc.vector.tensor_copy(
                            out=vO[:, h, tb, 0:HD],
                            in_=ps[:, 2 * DPC + h * HD : 2 * DPC + (h + 1) * HD],
                        )
                    # transpose q,k into [dim, token] layout
                    tp = t_ps.tile([128, 256], BF16, tag="tp")
                    nc.tensor.transpose(tp[:, 0:128], qn, eye_sb)
                    nc.tensor.transpose(tp[:, 128:256], kn, eye_sb)
                    ts = slice(tb * 128, (tb + 1) * 128)
                    nc.vector.tensor_copy(out=qT[:, ts], in_=tp[:, 0:128])
                    nc.scalar.copy(out=kT[:, ts], in_=tp[:, 128:256])

        # ---------------- Phase 2: attention ----------------------------
        with (
            tc.tile_pool(name="sc_ps", bufs=2, space="PSUM") as sc_ps,
            tc.tile_pool(name="o_ps", bufs=1, space="PSUM") as o_ps,
            tc.tile_pool(name="rb_ps", bufs=2, space="PSUM") as rb_ps,
            tc.tile_pool(name="exps", bufs=3) as exps,
            tc.tile_pool(name="stage2", bufs=3) as stage2,
            tc.tile_pool(name="ostage", bufs=4) as ostage,
        ):
            for b in range(B):
                for qc in range(QC):
                    cols = slice(b * S + qc * 512, b * S + (qc + 1) * 512)
                    oo = [
                        o_ps.tile([HD + 1, 512], F32, tag=f"o{h}", name=f"o{h}")
                        for h in range(HPC)
                    ]
                    for kb in range(KB):
                        gkb = b * KB + kb
                        ks = slice(gkb * 128, (gkb + 1) * 128)
                        # issue the two heads' score matmuls back-to-back:
                        # K=64 each at partition bases 0/64 -> disjoint row
                        # groups, so the PE runs them concurrently
                        # both heads' scores into one 2-bank PSUM tile so
                        # a single 1024-wide exp covers them (halves the ACT
                        # per-op overhead and the PE's exp-wait points)
                        scp = sc_ps.tile(
                            [128, HPC, 512], F32, tag="s", name="scp"
                        )
                        for h in range(HPC):
                            hp = slice(h * HD, (h + 1) * HD)
                            nc.tensor.matmul(
                                scp[:, h, :],
                                lhsT=kT[hp, ks],
                                rhs=qT[hp, cols],
                                start=True,
                                stop=True,
                            )
                        ex = exps.tile(
                            [128, HPC, 512], BF16, tag="ex", name="ex"
                        )
                        nc.scalar.activation(
                            out=ex, in_=scp, func=AF.Exp, scale=SCALE
                        )
                        for h in range(HPC):
                            nc.tensor.matmul(
                                oo[h],
                                lhsT=vO[:, h, gkb, :],
                                rhs=ex[:, h, :],
                                start=(kb == 0),
                                stop=(kb == KB - 1),
                            )
                    for h in range(HPC):
                        # 1/denom broadcast along 64 partitions without a
                        # (slow, single-lane) DVE reciprocal: ln(d) on the
                        # denom row, outer-product broadcast on the PE, then
                        # exp(-x) on ScalarE (same ACT table set as softmax).
                        rc = stage2.tile([1, 512], F32R, tag=f"rc{h}")
                        nc.scalar.activation(
                            out=rc, in_=oo[h][HD : HD + 1, :], func=AF.Ln
                        )
                        rb = rb_ps.tile([HD, 512], F32, tag="rb", name="rb")
                        nc.tensor.matmul(
                            rb,
                            lhsT=ones_sb[0:1, 0:HD],
                            rhs=rc,
                            start=True,
                            stop=True,
                        )
                        rbs = stage2.tile([HD, 512], F32, tag=f"rbs{h}")
                        nc.scalar.activation(
                            out=rbs, in_=rb, func=AF.Exp, scale=-1.0
                        )
                        nc.vector.tensor_mul(
                            aT[h * HD : (h + 1) * HD, cols], oo[h][0:HD, :], rbs
                        )
                    # fused partial projection for the 4 token blocks of
                    # this q-chunk (keeps the PE fed while the next chunk's
                    # exp tiles are produced; removes the separate proj tail)
                    for tbl in range(4):
                        tb = (b * QC + qc) * 4 + tbl
                        ob = ostage.tile([128, D], F32, tag="ob")
                        for nn in range(D // 512):
                            pps = rb_ps.tile(
                                [128, 512], F32, tag="rb", name="pps"
                            )
                            nc.tensor.matmul(
                                pps,
                                lhsT=onesb_sb[0:1, 0:128],
                                rhs=bp_sb[0:1, nn * 512 : (nn + 1) * 512],
                                start=True,
                                stop=False,
                            )
                            nc.tensor.matmul(
                                pps,
                                lhsT=aT[:, tb * 128 : (tb + 1) * 128],
                                rhs=wp_sb[:, nn * 512 : (nn + 1) * 512],
                                start=False,
                                stop=True,
                            )
                            if nn == 0:
                                nc.scalar.copy(
                                    out=ob[:, nn * 512 : (nn + 1) * 512],
                                    in_=pps,
                                )
                            else:
                                nc.vector.tensor_copy(
                                    out=ob[:, nn * 512 : (nn + 1) * 512],
                                    in_=pps,
                                )
                        nc.sync.dma_start(
                            out=outp[tb * 128 : (tb + 1) * 128, :], in_=ob
                        )

    nc.compile()
    return nc


def make_in_maps(x, w_qkv, b_qkv, w_proj, b_proj, q_gamma, q_beta, k_gamma, k_beta,
                 affine):
    B, S, _ = x.shape
    T = B * S
    xT = np.ascontiguousarray(x.reshape(T, D).T)
    ones = np.ones((1, 512), np.float32)
    eye = np.eye(128, dtype=np.float32)
    in_maps = []
    for c in range(NCORES):
        rs = slice(c * DPC, (c + 1) * DPC)
        w_slice = np.concatenate(
            [w_qkv[rs], w_qkv[D:2 * D][rs.start:rs.stop], w_qkv[2 * D:][rs.start:rs.stop]],
            axis=0,
        )  # [384, 1024]
        b_slice = np.concatenate(
            [b_qkv[rs], b_qkv[D:2 * D][rs.start:rs.stop], b_qkv[2 * D:][rs.start:rs.stop]]
        )[None, :]  # [1, 384]
        import ml_dtypes
        bf = ml_dtypes.bfloat16
        m = {
            "xT": xT.astype(bf),
            "wt_qkv": np.ascontiguousarray(w_slice.T).astype(bf),
            "b_qkv_s": np.ascontiguousarray(b_slice).astype(bf),
            "wt_proj": np.ascontiguousarray(w_proj[:, rs].T).astype(bf),
            "b_proj_s": (b_proj[None, :] if c == 0
                         else np.zeros((1, D), np.float32)).astype(bf),
            "c_ones": ones,
            "c_vones": np.ones((128, HPC, (T // 128), 1), bf),
            "c_onesb": np.ones((1, 512), bf),
            "c_eye": eye.astype(bf),
        }
        if affine:
            gb = np.stack([q_gamma, q_beta, k_gamma, k_beta])  # [4, 64]
            m["c_gb"] = np.ascontiguousarray(
                np.broadcast_to(gb[None], (128, 4, HD)).astype(np.float32)
            )
        in_maps.append(m)
    return in_maps


_NC_CACHE = {}

LAST_RESULTS = None


def kernel(x, w_qkv, b_qkv, w_proj, b_proj, q_gamma, q_beta, k_gamma, k_beta,
           **unused):
    global LAST_RESULTS
    x = np.asarray(x, np.float32)
    w_qkv = np.asarray(w_qkv, np.float32)
    b_qkv = np.asarray(b_qkv, np.float32)
    w_proj = np.asarray(w_proj, np.float32)
    b_proj = np.asarray(b_proj, np.float32)
    q_gamma = np.asarray(q_gamma, np.float32)
    q_beta = np.asarray(q_beta, np.float32)
    k_gamma = np.asarray(k_gamma, np.float32)
    k_beta = np.asarray(k_beta, np.float32)

    B, S, _ = x.shape
    affine = not (
        np.all(q_gamma == 1) and np.all(k_gamma == 1)
        and np.all(q_beta == 0) and np.all(k_beta == 0)
    )
    key = (B, S, affine)
    if key not in _NC_CACHE:
        _NC_CACHE[key] = build_nc(B, S, affine)
    nc = _NC_CACHE[key]

    in_maps = make_in_maps(
        x, w_qkv, b_qkv, w_proj, b_proj, q_gamma, q_beta, k_gamma, k_beta, affine
    )
    trace = bool(int(os.environ.get("BASS_KERNEL_TRACE", "0")))
    res = run_bass_kernel_spmd(
        nc, in_maps, core_ids=list(range(NCORES)), trace=trace
    )
    LAST_RESULTS = res
    acc = np.zeros((B * S, D), np.float32)
    for r in res.results:
        acc += r["outp"]
    return acc.reshape(B, S, D)


# revision 38
# speedup vs baseline: 1.1045x; 1.0111x over previous
"""Multi-head self-attention (B=2, S=2048, D=1024, H=16) on 8 TRN2 NeuronCores.

Tensor-parallel over heads: each core owns 2 heads. Accepts FULL inputs,
returns FULL output. Host pre-transposes x and slices per-head weights;
each core computes qkv -> per-head LayerNorm -> attention -> partial
output projection (over its 128 embed dims); host sums the 8 partials.
"""

import os
import sys

import numpy as np

for _p in ("/opt/trn_rl_repo", "/root/.axon_site/_ro/trn_rl_repo"):
    if os.path.isdir(_p) and _p not in sys.path:
        sys.path.insert(0, _p)
        break

import concourse.bass as bass  # noqa: E402
import concourse.bacc as bacc  # noqa: E402
import concourse.tile as tile  # noqa: E402
from concourse import mybir  # noqa: E402
from concourse.bass_utils import run_bass_kernel_spmd  # noqa: E402

F32 = mybir.dt.float32
F32R = mybir.dt.float32r
BF16 = mybir.dt.bfloat16
AF = mybir.ActivationFunctionType

NCORES = 8
D = 1024
H = 16
HD = 64
HPC = H // NCORES          # heads per core = 2
DPC = HPC * HD             # embed dims per core = 128
EPS = 1e-5


def _r(ap):
    return ap.bitcast(F32R)


def build_nc(B, S, affine):
    """Build the SPMD Bass program for one core (same program, 8 cores)."""
    T = B * S                      # total token columns
    NTB = T // 128                 # 128-token blocks
    NCH = T // 512                 # 512-token chunks
    QC = S // 512                  # q-chunks per batch
    KB = S // 128                  # k-blocks per batch
    KCH = D // 128                 # contraction chunks (8)
    SCALE = 1.0 / np.sqrt(HD)

    nc = bacc.Bacc(
        "TRN2",
        target_bir_lowering=False,
        debug=False,
        enable_asserts=True,
        num_devices=NCORES,
    )

    xT = nc.dram_tensor("xT", [D, T], BF16, kind="ExternalInput").ap()
    wq = nc.dram_tensor("wt_qkv", [D, 3 * DPC], BF16, kind="ExternalInput").ap()
    bq = nc.dram_tensor("b_qkv_s", [1, 3 * DPC], BF16, kind="ExternalInput").ap()
    wp = nc.dram_tensor("wt_proj", [DPC, D], BF16, kind="ExternalInput").ap()
    bp = nc.dram_tensor("b_proj_s", [1, D], BF16, kind="ExternalInput").ap()
    ones = nc.dram_tensor("c_ones", [1, 512], F32R, kind="ExternalInput").ap()
    vones = nc.dram_tensor(
        "c_vones", [128, HPC, NTB, 1], BF16, kind="ExternalInput"
    ).ap()
    onesb = nc.dram_tensor("c_onesb", [1, 512], BF16, kind="ExternalInput").ap()
    eye = nc.dram_tensor("c_eye", [128, 128], BF16, kind="ExternalInput").ap()
    if affine:
        gb = nc.dram_tensor("c_gb", [128, 4, HD], F32, kind="ExternalInput").ap()
    outp = nc.dram_tensor("outp", [T, D], F32, kind="ExternalOutput").ap()

    from contextlib import ExitStack

    with tile.TileContext(nc) as tc, ExitStack() as stack:
        const = stack.enter_context(tc.tile_pool(name="const", bufs=1))
        persist = stack.enter_context(tc.tile_pool(name="persist", bufs=1))

        wq_sb = const.tile([128, KCH, 3 * DPC], BF16, tag="wq")
        nc.sync.dma_start(
            out=wq_sb, in_=wq.rearrange("(c p) n -> p c n", p=128)
        )
        wp_sb = const.tile([DPC, D], BF16, tag="wp")
        nc.sync.dma_start(out=wp_sb, in_=wp)
        bq_sb = const.tile([1, 3 * DPC], BF16, tag="bq")
        nc.sync.dma_start(out=bq_sb, in_=bq)
        bp_sb = const.tile([1, D], BF16, tag="bp")
        nc.sync.dma_start(out=bp_sb, in_=bp)
        ones_sb = const.tile([1, 512], F32R, tag="ones")
        nc.sync.dma_start(out=ones_sb, in_=ones)
        onesb_sb = const.tile([1, 512], BF16, tag="onesb")
        nc.sync.dma_start(out=onesb_sb, in_=onesb)
        eye_sb = const.tile([128, 128], BF16, tag="eye")
        nc.sync.dma_start(out=eye_sb, in_=eye)
        eps_sb = const.tile([128, 1], F32, tag="eps")
        nc.vector.memset(eps_sb, EPS)

        if affine:
            gb_sb = const.tile([128, 4, HD], F32, tag="gb")
            nc.sync.dma_start(out=gb_sb, in_=gb)

        # persistent intermediates
        qT = persist.tile([128, T], BF16, tag="qT")     # [2h*64, tok] LN'd q^T
        kT = persist.tile([128, T], BF16, tag="kT")
        vO = persist.tile([128, HPC, NTB, HD + 1], BF16, tag="vO")  # v + ones col
        aT = persist.tile([128, T], BF16, tag="aT")     # attention out^T
        nc.sync.dma_start(out=vO[:, :, :, HD : HD + 1], in_=vones)

        # ---------------- Phase 1: qkv + LayerNorm + transpose ----------
        with (
            tc.tile_pool(name="xt", bufs=2) as xt_pool,
            tc.tile_pool(name="qkv_ps", bufs=4, space="PSUM") as qkv_ps,
            tc.tile_pool(name="t_ps", bufs=3, space="PSUM") as t_ps,
            tc.tile_pool(name="stage1", bufs=4) as stage1,
            tc.tile_pool(name="stats", bufs=4) as stats_pool,
        ):
            for n in range(NCH):
                xt = xt_pool.tile([128, KCH, 512], BF16, tag="xt")
                nc.sync.dma_start(
                    out=xt,
                    in_=xT.rearrange("(c p) t -> p c t", p=128)[
                        :, :, n * 512 : (n + 1) * 512
                    ],
                )
                for tbl in range(4):
                    tb = n * 4 + tbl
                    ps = qkv_ps.tile([128, 3 * DPC], F32, tag="ps")
                    nc.tensor.matmul(
                        ps,
                        lhsT=onesb_sb[0:1, 0:128],
                        rhs=bq_sb,
                        start=True,
                        stop=False,
                    )
                    for k in range(KCH):
                        nc.tensor.matmul(
                            ps,
                            lhsT=xt[:, k, tbl * 128 : (tbl + 1) * 128],
                            rhs=wq_sb[:, k, :],
                            start=False,
                            stop=(k == KCH - 1),
                        )
                    # LayerNorm over each head's 64 dims of q and k
                    qk = ps[:, 0 : 2 * DPC].rearrange("p (g d) -> p g d", d=HD)
                    st = stats_pool.tile([128, 4, 6], F32, tag="st")
                    mv = stats_pool.tile([128, 4, 2], F32, tag="mv")
                    for g in range(4):
                        nc.vector.bn_stats(out=st[:, g, :], in_=qk[:, g, :])
                        nc.vector.bn_aggr(out=mv[:, g, :], in_=st[:, g, :])
                    rstd = stats_pool.tile([128, 4], F32, tag="rstd")
                    nc.scalar.activation(
                        out=rstd, in_=mv[:, :, 1], func=AF.Sqrt, bias=eps_sb
                    )
                    nc.vector.reciprocal(out=rstd, in_=rstd)
                    qn = stage1.tile([128, 128], BF16, tag="qn")
                    kn = stage1.tile([128, 128], BF16, tag="kn")
                    for g in range(4):
                        dst = qn if g < 2 else kn
                        dsl = dst[:, (g % 2) * HD : (g % 2 + 1) * HD]
                        nc.vector.tensor_scalar(
                            out=dsl,
                            in0=qk[:, g, :],
                            scalar1=mv[:, g, 0:1],
                            scalar2=rstd[:, g : g + 1],
                            op0=mybir.AluOpType.subtract,
                            op1=mybir.AluOpType.mult,
                        )
                        if affine:
                            nc.vector.tensor_mul(dsl, dsl, gb_sb[:, 2 * (g // 2), :])
                            nc.vector.tensor_add(
                                dsl, dsl, gb_sb[:, 2 * (g // 2) + 1, :]
                            )
                    # v (+ ones col already set)
                    for h in range(HPC):
                        nc.vector.tensor_copy(
                            out=vO[:, h, tb, 0:HD],
                            in_=ps[:, 2 * DPC + h * HD : 2 * DPC + (h + 1) * HD],
                        )
                    # transpose q,k into [dim, token] layout
                    tp = t_ps.tile([128, 256], BF16, tag="tp")
                    nc.tensor.transpose(tp[:, 0:128], qn, eye_sb)
                    nc.tensor.transpose(tp[:, 128:256], kn, eye_sb)
                    ts = slice(tb * 128, (tb + 1) * 128)
                    nc.vector.tensor_copy(out=qT[:, ts], in_=tp[:, 0:128])
                    nc.scalar.copy(out=kT[:, ts], in_=tp[:, 128:256])

        # ---------------- Phase 2: attention ----------------------------
        with (
            tc.tile_pool(name="sc_ps", bufs=2, space="PSUM") as sc_ps,
            tc.tile_pool(name="o_ps", bufs=1, space="PSUM") as o_ps,
            tc.tile_pool(name="rb_ps", bufs=2, space="PSUM") as rb_ps,
            tc.tile_pool(name="exps", bufs=3) as exps,
            tc.tile_pool(name="stage2", bufs=3) as stage2,
            tc.tile_pool(name="ostage", bufs=4) as ostage,
        ):
            for b in range(B):
                for qc in range(QC):
                    cols = slice(b * S + qc * 512, b * S + (qc + 1) * 512)
                    oo = [
                        o_ps.tile([HD + 1, 512], F32, tag=f"o{h}", name=f"o{h}")
                        for h in range(HPC)
                    ]
                    for kb in range(KB):
                        gkb = b * KB + kb
                        ks = slice(gkb * 128, (gkb + 1) * 128)
                        # issue the two heads' score matmuls back-to-back:
                        # K=64 each at partition bases 0/64 -> disjoint row
                        # groups, so the PE runs them concurrently
                        # both heads' scores into one 2-bank PSUM tile so
                        # a single 1024-wide exp covers them (halves the ACT
                        # per-op overhead and the PE's exp-wait points)
                        scp = sc_ps.tile(
                            [128, HPC, 512], F32, tag="s", name="scp"
                        )
                        for h in range(HPC):
                            hp = slice(h * HD, (h + 1) * HD)
                            nc.tensor.matmul(
                                scp[:, h, :],
                                lhsT=kT[hp, ks],
                                rhs=qT[hp, cols],
                                start=True,
                                stop=True,
                            )
                        ex = exps.tile(
                            [128, HPC, 512], BF16, tag="ex", name="ex"
                        )
                        nc.scalar.activation(
                            out=ex, in_=scp, func=AF.Exp, scale=SCALE
                        )
                        for h in range(HPC):
                            nc.tensor.matmul(
                                oo[h],
                                lhsT=vO[:, h, gkb, :],
                                rhs=ex[:, h, :],
                                start=(kb == 0),
                                stop=(kb == KB - 1),
                            )
                    for h in range(HPC):
                        # 1/denom broadcast along 64 partitions without a
                        # (slow, single-lane) DVE reciprocal: ln(d) on the
                        # denom row, outer-product broadcast on the PE, then
                        # exp(-x) on ScalarE (same ACT table set as softmax).
                        rc = stage2.tile([1, 512], F32R, tag=f"rc{h}")
                        nc.scalar.activation(
                            out=rc, in_=oo[h][HD : HD + 1, :], func=AF.Ln
                        )
                        rb = rb_ps.tile([HD, 512], F32, tag="rb", name="rb")
                        nc.tensor.matmul(
                            rb,
                            lhsT=ones_sb[0:1, 0:HD],
                            rhs=rc,
                            start=True,
                            stop=True,
                        )
                        rbs = stage2.tile([HD, 512], F32, tag=f"rbs{h}")
                        nc.scalar.activation(
                            out=rbs, in_=rb, func=AF.Exp, scale=-1.0
                        )
                        nc.vector.tensor_mul(
                            aT[h * HD : (h + 1) * HD, cols], oo[h][0:HD, :], rbs
                        )
                    # fused partial projection for the 4 token blocks of
                    # this q-chunk (keeps the PE fed while the next chunk's
                    # exp tiles are produced; removes the separate proj tail)
                    for tbl in range(4):
                        tb = (b * QC + qc) * 4 + tbl
                        ob = ostage.tile([128, D], F32, tag="ob")
                        for nn in range(D // 512):
                            pps = rb_ps.tile(
                                [128, 512], F32, tag="rb", name="pps"
                            )
                            nc.tensor.matmul(
                                pps,
                                lhsT=onesb_sb[0:1, 0:128],
                                rhs=bp_sb[0:1, nn * 512 : (nn + 1) * 512],
                                start=True,
                                stop=False,
                            )
                            nc.tensor.matmul(
                                pps,
                                lhsT=aT[:, tb * 128 : (tb + 1) * 128],
                                rhs=wp_sb[:, nn * 512 : (nn + 1) * 512],
                                start=False,
                                stop=True,
                            )
                            if nn == 0:
                                nc.scalar.copy(
                                    out=ob[:, nn * 512 : (nn + 1) * 512],
                                    in_=pps,
                                )
                            else:
                                nc.vector.tensor_copy(
                                    out=ob[:, nn * 512 : (nn + 1) * 512],
                                    in_=pps,
                                )
                        nc.sync.dma_start(
                            out=outp[tb * 128 : (tb + 1) * 128, :], in_=ob
                        )

    nc.compile()
    return nc


def make_in_maps(x, w_qkv, b_qkv, w_proj, b_proj, q_gamma, q_beta, k_gamma, k_beta,
                 affine):
    B, S, _ = x.shape
    T = B * S
    xT = np.ascontiguousarray(x.reshape(T, D).T)
    ones = np.ones((1, 512), np.float32)
    eye = np.eye(128, dtype=np.float32)
    in_maps = []
    for c in range(NCORES):
        rs = slice(c * DPC, (c + 1) * DPC)
        w_slice = np.concatenate(
            [w_qkv[rs], w_qkv[D:2 * D][rs.start:rs.stop], w_qkv[2 * D:][rs.start:rs.stop]],
            axis=0,
        )  # [384, 1024]
        b_slice = np.concatenate(
            [b_qkv[rs], b_qkv[D:2 * D][rs.start:rs.stop], b_qkv[2 * D:][rs.start:rs.stop]]
        )[None, :]  # [1, 384]
        import ml_dtypes
        bf = ml_dtypes.bfloat16
        m = {
            "xT": xT.astype(bf),
            "wt_qkv": np.ascontiguousarray(w_slice.T).astype(bf),
            "b_qkv_s": np.ascontiguousarray(b_slice).astype(bf),
            "wt_proj": np.ascontiguousarray(w_proj[:, rs].T).astype(bf),
            "b_proj_s": (b_proj[None, :] if c == 0
                         else np.zeros((1, D), np.float32)).astype(bf),
            "c_ones": ones,
            "c_vones": np.ones((128, HPC, (T // 128), 1), bf),
            "c_onesb": np.ones((1, 512), bf),
            "c_eye": eye.astype(bf),
        }
        if affine:
            gb = np.stack([q_gamma, q_beta, k_gamma, k_beta])  # [4, 64]
            m["c_gb"] = np.ascontiguousarray(
                np.broadcast_to(gb[None], (128, 4, HD)).astype(np.float32)
            )
        in_maps.append(m)
    return in_maps


_NC_CACHE = {}

LAST_RESULTS = None


def kernel(x, w_qkv, b_qkv, w_proj, b_proj, q_gamma, q_beta, k_gamma, k_beta,
           **unused):
    global LAST_RESULTS
    x = np.asarray(x, np.float32)
    w_qkv = np.asarray(w_qkv, np.float32)
    b_qkv = np.asarray(b_qkv, np.float32)
    w_proj = np.asarray(w_proj, np.float32)
    b_proj = np.asarray(b_proj, np.float32)
    q_gamma = np.asarray(q_gamma, np.float32)
    q_beta = np.asarray(q_beta, np.float32)
    k_gamma = np.asarray(k_gamma, np.float32)
    k_beta = np.asarray(k_beta, np.float32)

    B, S, _ = x.shape
    affine = not (
        np.all(q_gamma == 1) and np.all(k_gamma == 1)
        and np.all(q_beta == 0) and np.all(k_beta == 0)
    )
    key = (B, S, affine)
    if key not in _NC_CACHE:
        _NC_CACHE[key] = build_nc(B, S, affine)
    nc = _NC_CACHE[key]

    in_maps = make_in_maps(
        x, w_qkv, b_qkv, w_proj, b_proj, q_gamma, q_beta, k_gamma, k_beta, affine
    )
    trace = bool(int(os.environ.get("BASS_KERNEL_TRACE", "0")))
    res = run_bass_kernel_spmd(
        nc, in_maps, core_ids=list(range(NCORES)), trace=trace
    )
    LAST_RESULTS = res
    acc = np.zeros((B * S, D), np.float32)
    for r in res.results:
        acc += r["outp"]
    return acc.reshape(B, S, D)


# revision 39
# speedup vs baseline: 1.2493x; 1.1311x over previous
"""Multi-head self-attention (B=2, S=2048, D=1024, H=16) on 8 TRN2 NeuronCores.

Tensor-parallel over heads: each core owns 2 heads. Accepts FULL inputs,
returns FULL output. Host pre-transposes x and slices per-head weights;
each core computes qkv -> per-head LayerNorm -> attention -> partial
output projection (over its 128 embed dims); host sums the 8 partials.
"""

import os
import sys

import numpy as np

for _p in ("/opt/trn_rl_repo", "/root/.axon_site/_ro/trn_rl_repo"):
    if os.path.isdir(_p) and _p not in sys.path:
        sys.path.insert(0, _p)
        break

import concourse.bass as bass  # noqa: E402
import concourse.bacc as bacc  # noqa: E402
import concourse.tile as tile  # noqa: E402
from concourse import mybir  # noqa: E402
from concourse.bass_utils import run_bass_kernel_spmd  # noqa: E402

F32 = mybir.dt.float32
F32R = mybir.dt.float32r
BF16 = mybir.dt.bfloat16
AF = mybir.ActivationFunctionType

NCORES = 8
D = 1024
H = 16
HD = 64
HPC = H // NCORES          # heads per core = 2
DPC = HPC * HD             # embed dims per core = 128
EPS = 1e-5


def _r(ap):
    return ap.bitcast(F32R)


def build_nc(B, S, affine):
    """Build the SPMD Bass program for one core (same program, 8 cores)."""
    T = B * S                      # total token columns
    NTB = T // 128                 # 128-token blocks
    NCH = T // 512                 # 512-token chunks
    QC = S // 512                  # q-chunks per batch
    KB = S // 128                  # k-blocks per batch
    KCH = D // 128                 # contraction chunks (8)
    SCALE = 1.0 / np.sqrt(HD)

    nc = bacc.Bacc(
        "TRN2",
        target_bir_lowering=False,
        debug=False,
        enable_asserts=True,
        num_devices=NCORES,
    )

    xT = nc.dram_tensor("xT", [D, T], BF16, kind="ExternalInput").ap()
    wq = nc.dram_tensor("wt_qkv", [D, 3 * DPC], BF16, kind="ExternalInput").ap()
    bq = nc.dram_tensor("b_qkv_s", [1, 3 * DPC], BF16, kind="ExternalInput").ap()
    wp = nc.dram_tensor("wt_proj", [DPC, D], BF16, kind="ExternalInput").ap()
    bpb = nc.dram_tensor("c_bpb", [128, D], F32, kind="ExternalInput").ap()
    ones = nc.dram_tensor("c_ones", [1, 512], F32R, kind="ExternalInput").ap()
    vones = nc.dram_tensor(
        "c_vones", [128, HPC, NTB, 1], BF16, kind="ExternalInput"
    ).ap()
    onesb = nc.dram_tensor("c_onesb", [1, 512], BF16, kind="ExternalInput").ap()
    eye = nc.dram_tensor("c_eye", [128, 128], BF16, kind="ExternalInput").ap()
    if affine:
        gb = nc.dram_tensor("c_gb", [128, 4, HD], F32, kind="ExternalInput").ap()
    outp = nc.dram_tensor("outp", [T, D], F32, kind="ExternalOutput").ap()

    from contextlib import ExitStack

    with tile.TileContext(nc) as tc, ExitStack() as stack:
        const = stack.enter_context(tc.tile_pool(name="const", bufs=1))
        persist = stack.enter_context(tc.tile_pool(name="persist", bufs=1))

        wq_sb = const.tile([128, KCH, 3 * DPC], BF16, tag="wq")
        nc.sync.dma_start(
            out=wq_sb, in_=wq.rearrange("(c p) n -> p c n", p=128)
        )
        wp_sb = const.tile([DPC, D], BF16, tag="wp")
        nc.sync.dma_start(out=wp_sb, in_=wp)
        bq_sb = const.tile([1, 3 * DPC], BF16, tag="bq")
        nc.sync.dma_start(out=bq_sb, in_=bq)
        bpb_sb = const.tile([128, D], F32, tag="bpb")
        nc.sync.dma_start(out=bpb_sb, in_=bpb)
        ones_sb = const.tile([1, 512], F32R, tag="ones")
        nc.sync.dma_start(out=ones_sb, in_=ones)
        onesb_sb = const.tile([1, 512], BF16, tag="onesb")
        nc.sync.dma_start(out=onesb_sb, in_=onesb)
        eye_sb = const.tile([128, 128], BF16, tag="eye")
        nc.sync.dma_start(out=eye_sb, in_=eye)
        eps_sb = const.tile([128, 1], F32, tag="eps")
        nc.vector.memset(eps_sb, EPS)

        if affine:
            gb_sb = const.tile([128, 4, HD], F32, tag="gb")
            nc.sync.dma_start(out=gb_sb, in_=gb)

        # persistent intermediates
        qT = persist.tile([128, T], BF16, tag="qT")     # [2h*64, tok] LN'd q^T
        kT = persist.tile([128, T], BF16, tag="kT")
        vO = persist.tile([128, HPC, NTB, HD + 1], BF16, tag="vO")  # v + ones col
        aT = persist.tile([128, T], BF16, tag="aT")     # attention out^T
        nc.sync.dma_start(out=vO[:, :, :, HD : HD + 1], in_=vones)

        # ---------------- Phase 1: qkv + LayerNorm + transpose ----------
        with (
            tc.tile_pool(name="xt", bufs=2) as xt_pool,
            tc.tile_pool(name="qkv_ps", bufs=4, space="PSUM") as qkv_ps,
            tc.tile_pool(name="t_ps", bufs=3, space="PSUM") as t_ps,
            tc.tile_pool(name="stage1", bufs=4) as stage1,
            tc.tile_pool(name="stats", bufs=4) as stats_pool,
        ):
            for n in range(NCH):
                xt = xt_pool.tile([128, KCH, 512], BF16, tag="xt")
                nc.sync.dma_start(
                    out=xt,
                    in_=xT.rearrange("(c p) t -> p c t", p=128)[
                        :, :, n * 512 : (n + 1) * 512
                    ],
                )
                for tbl in range(4):
                    tb = n * 4 + tbl
                    ps = qkv_ps.tile([128, 3 * DPC], F32, tag="ps")
                    nc.tensor.matmul(
                        ps,
                        lhsT=onesb_sb[0:1, 0:128],
                        rhs=bq_sb,
                        start=True,
                        stop=False,
                    )
                    for k in range(KCH):
                        nc.tensor.matmul(
                            ps,
                            lhsT=xt[:, k, tbl * 128 : (tbl + 1) * 128],
                            rhs=wq_sb[:, k, :],
                            start=False,
                            stop=(k == KCH - 1),
                        )
                    # LayerNorm over each head's 64 dims of q and k
                    qk = ps[:, 0 : 2 * DPC].rearrange("p (g d) -> p g d", d=HD)
                    st = stats_pool.tile([128, 4, 6], F32, tag="st")
                    mv = stats_pool.tile([128, 4, 2], F32, tag="mv")
                    for g in range(4):
                        nc.vector.bn_stats(out=st[:, g, :], in_=qk[:, g, :])
                        nc.vector.bn_aggr(out=mv[:, g, :], in_=st[:, g, :])
                    rstd = stats_pool.tile([128, 4], F32, tag="rstd")
                    nc.scalar.activation(
                        out=rstd, in_=mv[:, :, 1], func=AF.Sqrt, bias=eps_sb
                    )
                    nc.vector.reciprocal(out=rstd, in_=rstd)
                    qn = stage1.tile([128, 128], BF16, tag="qn")
                    kn = stage1.tile([128, 128], BF16, tag="kn")
                    for g in range(4):
                        dst = qn if g < 2 else kn
                        dsl = dst[:, (g % 2) * HD : (g % 2 + 1) * HD]
                        nc.vector.tensor_scalar(
                            out=dsl,
                            in0=qk[:, g, :],
                            scalar1=mv[:, g, 0:1],
                            scalar2=rstd[:, g : g + 1],
                            op0=mybir.AluOpType.subtract,
                            op1=mybir.AluOpType.mult,
                        )
                        if affine:
                            nc.vector.tensor_mul(dsl, dsl, gb_sb[:, 2 * (g // 2), :])
                            nc.vector.tensor_add(
                                dsl, dsl, gb_sb[:, 2 * (g // 2) + 1, :]
                            )
                    # v (+ ones col already set)
                    for h in range(HPC):
                        nc.vector.tensor_copy(
                            out=vO[:, h, tb, 0:HD],
                            in_=ps[:, 2 * DPC + h * HD : 2 * DPC + (h + 1) * HD],
                        )
                    # transpose q,k into [dim, token] layout
                    tp = t_ps.tile([128, 256], BF16, tag="tp")
                    nc.tensor.transpose(tp[:, 0:128], qn, eye_sb)
                    nc.tensor.transpose(tp[:, 128:256], kn, eye_sb)
                    ts = slice(tb * 128, (tb + 1) * 128)
                    nc.vector.tensor_copy(out=qT[:, ts], in_=tp[:, 0:128])
                    nc.scalar.copy(out=kT[:, ts], in_=tp[:, 128:256])

        # ---------------- Phase 2: attention ----------------------------
        with (
            tc.tile_pool(name="sc_ps", bufs=2, space="PSUM") as sc_ps,
            tc.tile_pool(name="o_ps", bufs=1, space="PSUM") as o_ps,
            tc.tile_pool(name="rb_ps", bufs=2, space="PSUM") as rb_ps,
            tc.tile_pool(name="exps", bufs=3) as exps,
            tc.tile_pool(name="stage2", bufs=3) as stage2,
            tc.tile_pool(name="ostage", bufs=4) as ostage,
        ):
            for b in range(B):
                for qc in range(QC):
                    cols = slice(b * S + qc * 512, b * S + (qc + 1) * 512)
                    oo = [
                        o_ps.tile([HD + 1, 512], F32, tag=f"o{h}", name=f"o{h}")
                        for h in range(HPC)
                    ]
                    for kb in range(KB):
                        gkb = b * KB + kb
                        ks = slice(gkb * 128, (gkb + 1) * 128)
                        # issue the two heads' score matmuls back-to-back:
                        # K=64 each at partition bases 0/64 -> disjoint row
                        # groups, so the PE runs them concurrently
                        # both heads' scores into one 2-bank PSUM tile so
                        # a single 1024-wide exp covers them (halves the ACT
                        # per-op overhead and the PE's exp-wait points)
                        scp = sc_ps.tile(
                            [128, HPC, 512], F32, tag="s", name="scp"
                        )
                        for h in range(HPC):
                            hp = slice(h * HD, (h + 1) * HD)
                            nc.tensor.matmul(
                                scp[:, h, :],
                                lhsT=kT[hp, ks],
                                rhs=qT[hp, cols],
                                start=True,
                                stop=True,
                            )
                        ex = exps.tile(
                            [128, HPC, 512], BF16, tag="ex", name="ex"
                        )
                        nc.scalar.activation(
                            out=ex, in_=scp, func=AF.Exp, scale=SCALE
                        )
                        for h in range(HPC):
                            nc.tensor.matmul(
                                oo[h],
                                lhsT=vO[:, h, gkb, :],
                                rhs=ex[:, h, :],
                                start=(kb == 0),
                                stop=(kb == KB - 1),
                            )
                    for h in range(HPC):
                        # 1/denom broadcast along 64 partitions without a
                        # (slow, single-lane) DVE reciprocal: ln(d) on the
                        # denom row, outer-product broadcast on the PE, then
                        # exp(-x) on ScalarE (same ACT table set as softmax).
                        rc = stage2.tile([1, 512], F32R, tag=f"rc{h}")
                        nc.scalar.activation(
                            out=rc, in_=oo[h][HD : HD + 1, :], func=AF.Ln
                        )
                        rb = rb_ps.tile([HD, 512], F32, tag="rb", name="rb")
                        nc.tensor.matmul(
                            rb,
                            lhsT=ones_sb[0:1, 0:HD],
                            rhs=rc,
                            start=True,
                            stop=True,
                        )
                        rbs = stage2.tile([HD, 512], F32, tag=f"rbs{h}")
                        nc.scalar.activation(
                            out=rbs, in_=rb, func=AF.Exp, scale=-1.0
                        )
                        nc.vector.tensor_mul(
                            aT[h * HD : (h + 1) * HD, cols], oo[h][0:HD, :], rbs
                        )
                    # fused partial projection for the 4 token blocks of
                    # this q-chunk (keeps the PE fed while the next chunk's
                    # exp tiles are produced; removes the separate proj tail)
                    for tbl in range(4):
                        tb = (b * QC + qc) * 4 + tbl
                        ob = ostage.tile([128, D], F32, tag="ob")
                        for nn in range(D // 512):
                            pps = rb_ps.tile(
                                [128, 512], F32, tag="rb", name="pps"
                            )
                            nc.tensor.matmul(
                                pps,
                                lhsT=aT[:, tb * 128 : (tb + 1) * 128],
                                rhs=wp_sb[:, nn * 512 : (nn + 1) * 512],
                                start=True,
                                stop=True,
                            )
                            # bias folded into the eviction (bias was pre-
                            # broadcast across partitions on the host)
                            nc.vector.tensor_add(
                                ob[:, nn * 512 : (nn + 1) * 512],
                                pps,
                                bpb_sb[:, nn * 512 : (nn + 1) * 512],
                            )
                        nc.sync.dma_start(
                            out=outp[tb * 128 : (tb + 1) * 128, :], in_=ob
                        )

    nc.compile()
    return nc


def make_in_maps(x, w_qkv, b_qkv, w_proj, b_proj, q_gamma, q_beta, k_gamma, k_beta,
                 affine):
    B, S, _ = x.shape
    T = B * S
    xT = np.ascontiguousarray(x.reshape(T, D).T)
    ones = np.ones((1, 512), np.float32)
    eye = np.eye(128, dtype=np.float32)
    in_maps = []
    for c in range(NCORES):
        rs = slice(c * DPC, (c + 1) * DPC)
        w_slice = np.concatenate(
            [w_qkv[rs], w_qkv[D:2 * D][rs.start:rs.stop], w_qkv[2 * D:][rs.start:rs.stop]],
            axis=0,
        )  # [384, 1024]
        b_slice = np.concatenate(
            [b_qkv[rs], b_qkv[D:2 * D][rs.start:rs.stop], b_qkv[2 * D:][rs.start:rs.stop]]
        )[None, :]  # [1, 384]
        import ml_dtypes
        bf = ml_dtypes.bfloat16
        m = {
            "xT": xT.astype(bf),
            "wt_qkv": np.ascontiguousarray(w_slice.T).astype(bf),
            "b_qkv_s": np.ascontiguousarray(b_slice).astype(bf),
            "wt_proj": np.ascontiguousarray(w_proj[:, rs].T).astype(bf),
            "c_bpb": np.ascontiguousarray(np.broadcast_to(
                b_proj[None, :] if c == 0 else np.zeros((1, D), np.float32),
                (128, D))).astype(np.float32),
            "c_ones": ones,
            "c_vones": np.ones((128, HPC, (T // 128), 1), bf),
            "c_onesb": np.ones((1, 512), bf),
            "c_eye": eye.astype(bf),
        }
        if affine:
            gb = np.stack([q_gamma, q_beta, k_gamma, k_beta])  # [4, 64]
            m["c_gb"] = np.ascontiguousarray(
                np.broadcast_to(gb[None], (128, 4, HD)).astype(np.float32)
            )
        in_maps.append(m)
    return in_maps


_NC_CACHE = {}

LAST_RESULTS = None


def kernel(x, w_qkv, b_qkv, w_proj, b_proj, q_gamma, q_beta, k_gamma, k_beta,
           **unused):
    global LAST_RESULTS
    x = np.asarray(x, np.float32)
    w_qkv = np.asarray(w_qkv, np.float32)
    b_qkv = np.asarray(b_qkv, np.float32)
    w_proj = np.asarray(w_proj, np.float32)
    b_proj = np.asarray(b_proj, np.float32)
    q_gamma = np.asarray(q_gamma, np.float32)
    q_beta = np.asarray(q_beta, np.float32)
    k_gamma = np.asarray(k_gamma, np.float32)
    k_beta = np.asarray(k_beta, np.float32)

    B, S, _ = x.shape
    affine = not (
        np.all(q_gamma == 1) and np.all(k_gamma == 1)
        and np.all(q_beta == 0) and np.all(k_beta == 0)
    )
    key = (B, S, affine)
    if key not in _NC_CACHE:
        _NC_CACHE[key] = build_nc(B, S, affine)
    nc = _NC_CACHE[key]

    in_maps = make_in_maps(
        x, w_qkv, b_qkv, w_proj, b_proj, q_gamma, q_beta, k_gamma, k_beta, affine
    )
    trace = bool(int(os.environ.get("BASS_KERNEL_TRACE", "0")))
    res = run_bass_kernel_spmd(
        nc, in_maps, core_ids=list(range(NCORES)), trace=trace
    )
    LAST_RESULTS = res
    acc = np.zeros((B * S, D), np.float32)
    for r in res.results:
        acc += r["outp"]
    return acc.reshape(B, S, D)


# revision 40
# speedup vs baseline: 1.2533x; 1.0032x over previous
"""Multi-head self-attention (B=2, S=2048, D=1024, H=16) on 8 TRN2 NeuronCores.

Tensor-parallel over heads: each core owns 2 heads. Accepts FULL inputs,
returns FULL output. Host pre-transposes x and slices per-head weights;
each core computes qkv -> per-head LayerNorm -> attention -> partial
output projection (over its 128 embed dims); host sums the 8 partials.
"""

import os
import sys

import numpy as np

for _p in ("/opt/trn_rl_repo", "/root/.axon_site/_ro/trn_rl_repo"):
    if os.path.isdir(_p) and _p not in sys.path:
        sys.path.insert(0, _p)
        break

import concourse.bass as bass  # noqa: E402
import concourse.bacc as bacc  # noqa: E402
import concourse.tile as tile  # noqa: E402
from concourse import mybir  # noqa: E402
from concourse.bass_utils import run_bass_kernel_spmd  # noqa: E402

F32 = mybir.dt.float32
F32R = mybir.dt.float32r
BF16 = mybir.dt.bfloat16
AF = mybir.ActivationFunctionType

NCORES = 8
D = 1024
H = 16
HD = 64
HPC = H // NCORES          # heads per core = 2
DPC = HPC * HD             # embed dims per core = 128
EPS = 1e-5


def _r(ap):
    return ap.bitcast(F32R)


def build_nc(B, S, affine):
    """Build the SPMD Bass program for one core (same program, 8 cores)."""
    T = B * S                      # total token columns
    NTB = T // 128                 # 128-token blocks
    NCH = T // 512                 # 512-token chunks
    QC = S // 512                  # q-chunks per batch
    KB = S // 128                  # k-blocks per batch
    KCH = D // 128                 # contraction chunks (8)
    SCALE = 1.0 / np.sqrt(HD)

    nc = bacc.Bacc(
        "TRN2",
        target_bir_lowering=False,
        debug=False,
        enable_asserts=True,
        num_devices=NCORES,
    )

    xT = nc.dram_tensor("xT", [D, T], BF16, kind="ExternalInput").ap()
    wq = nc.dram_tensor("wt_qkv", [D, 3 * DPC], BF16, kind="ExternalInput").ap()
    bq = nc.dram_tensor("b_qkv_s", [1, 3 * DPC], BF16, kind="ExternalInput").ap()
    wp = nc.dram_tensor("wt_proj", [DPC, D], BF16, kind="ExternalInput").ap()
    bpb = nc.dram_tensor("c_bpb", [128, D], F32, kind="ExternalInput").ap()
    ones = nc.dram_tensor("c_ones", [1, 512], F32R, kind="ExternalInput").ap()
    vones = nc.dram_tensor(
        "c_vones", [128, HPC, NTB, 1], BF16, kind="ExternalInput"
    ).ap()
    onesb = nc.dram_tensor("c_onesb", [1, 512], BF16, kind="ExternalInput").ap()
    eye = nc.dram_tensor("c_eye", [128, 128], BF16, kind="ExternalInput").ap()
    if affine:
        gb = nc.dram_tensor("c_gb", [128, 4, HD], F32, kind="ExternalInput").ap()
    outp = nc.dram_tensor("outp", [T, D], F32, kind="ExternalOutput").ap()

    from contextlib import ExitStack

    with tile.TileContext(nc) as tc, ExitStack() as stack:
        const = stack.enter_context(tc.tile_pool(name="const", bufs=1))
        persist = stack.enter_context(tc.tile_pool(name="persist", bufs=1))

        wq_sb = const.tile([128, KCH, 3 * DPC], BF16, tag="wq")
        nc.sync.dma_start(
            out=wq_sb, in_=wq.rearrange("(c p) n -> p c n", p=128)
        )
        wp_sb = const.tile([DPC, D], BF16, tag="wp")
        nc.sync.dma_start(out=wp_sb, in_=wp)
        bq_sb = const.tile([1, 3 * DPC], BF16, tag="bq")
        nc.sync.dma_start(out=bq_sb, in_=bq)
        bpb_sb = const.tile([128, D], F32, tag="bpb")
        nc.sync.dma_start(out=bpb_sb, in_=bpb)
        ones_sb = const.tile([1, 512], F32R, tag="ones")
        nc.sync.dma_start(out=ones_sb, in_=ones)
        onesb_sb = const.tile([1, 512], BF16, tag="onesb")
        nc.sync.dma_start(out=onesb_sb, in_=onesb)
        eye_sb = const.tile([128, 128], BF16, tag="eye")
        nc.sync.dma_start(out=eye_sb, in_=eye)
        eps_sb = const.tile([128, 1], F32, tag="eps")
        nc.vector.memset(eps_sb, EPS)

        if affine:
            gb_sb = const.tile([128, 4, HD], F32, tag="gb")
            nc.sync.dma_start(out=gb_sb, in_=gb)

        # persistent intermediates
        qT = persist.tile([128, T], BF16, tag="qT")     # [2h*64, tok] LN'd q^T
        kT = persist.tile([128, T], BF16, tag="kT")
        vO = persist.tile([128, HPC, NTB, HD + 1], BF16, tag="vO")  # v + ones col
        aT = persist.tile([128, T], BF16, tag="aT")     # attention out^T
        nc.sync.dma_start(out=vO[:, :, :, HD : HD + 1], in_=vones)

        # ---------------- Phase 1: qkv + LayerNorm + transpose ----------
        with (
            tc.tile_pool(name="xt", bufs=2) as xt_pool,
            tc.tile_pool(name="qkv_ps", bufs=4, space="PSUM") as qkv_ps,
            tc.tile_pool(name="t_ps", bufs=3, space="PSUM") as t_ps,
            tc.tile_pool(name="stage1", bufs=4) as stage1,
            tc.tile_pool(name="stats", bufs=4) as stats_pool,
        ):
            for n in range(NCH):
                xt = xt_pool.tile([128, KCH, 512], BF16, tag="xt")
                nc.sync.dma_start(
                    out=xt,
                    in_=xT.rearrange("(c p) t -> p c t", p=128)[
                        :, :, n * 512 : (n + 1) * 512
                    ],
                )
                for tbl in range(4):
                    tb = n * 4 + tbl
                    ps = qkv_ps.tile([128, 3 * DPC], F32, tag="ps")
                    nc.tensor.matmul(
                        ps,
                        lhsT=onesb_sb[0:1, 0:128],
                        rhs=bq_sb,
                        start=True,
                        stop=False,
                    )
                    for k in range(KCH):
                        nc.tensor.matmul(
                            ps,
                            lhsT=xt[:, k, tbl * 128 : (tbl + 1) * 128],
                            rhs=wq_sb[:, k, :],
                            start=False,
                            stop=(k == KCH - 1),
                        )
                    # LayerNorm over each head's 64 dims of q and k
                    qk = ps[:, 0 : 2 * DPC].rearrange("p (g d) -> p g d", d=HD)
                    st = stats_pool.tile([128, 4, 6], F32, tag="st")
                    mv = stats_pool.tile([128, 4, 2], F32, tag="mv")
                    for g in range(4):
                        nc.vector.bn_stats(out=st[:, g, :], in_=qk[:, g, :])
                        nc.vector.bn_aggr(out=mv[:, g, :], in_=st[:, g, :])
                    rstd = stats_pool.tile([128, 4], F32, tag="rstd")
                    nc.scalar.activation(
                        out=rstd, in_=mv[:, :, 1], func=AF.Sqrt, bias=eps_sb
                    )
                    nc.vector.reciprocal(out=rstd, in_=rstd)
                    qn = stage1.tile([128, 128], BF16, tag="qn")
                    kn = stage1.tile([128, 128], BF16, tag="kn")
                    for g in range(4):
                        dst = qn if g < 2 else kn
                        dsl = dst[:, (g % 2) * HD : (g % 2 + 1) * HD]
                        nc.vector.tensor_scalar(
                            out=dsl,
                            in0=qk[:, g, :],
                            scalar1=mv[:, g, 0:1],
                            scalar2=rstd[:, g : g + 1],
                            op0=mybir.AluOpType.subtract,
                            op1=mybir.AluOpType.mult,
                        )
                        if affine:
                            nc.vector.tensor_mul(dsl, dsl, gb_sb[:, 2 * (g // 2), :])
                            nc.vector.tensor_add(
                                dsl, dsl, gb_sb[:, 2 * (g // 2) + 1, :]
                            )
                    # v (+ ones col already set)
                    for h in range(HPC):
                        nc.vector.tensor_copy(
                            out=vO[:, h, tb, 0:HD],
                            in_=ps[:, 2 * DPC + h * HD : 2 * DPC + (h + 1) * HD],
                        )
                    # transpose q,k into [dim, token] layout
                    tp = t_ps.tile([128, 256], BF16, tag="tp")
                    nc.tensor.transpose(tp[:, 0:128], qn, eye_sb)
                    nc.tensor.transpose(tp[:, 128:256], kn, eye_sb)
                    ts = slice(tb * 128, (tb + 1) * 128)
                    nc.vector.tensor_copy(out=qT[:, ts], in_=tp[:, 0:128])
                    nc.scalar.copy(out=kT[:, ts], in_=tp[:, 128:256])

        # ---------------- Phase 2: attention ----------------------------
        with (
            tc.tile_pool(name="sc_ps", bufs=2, space="PSUM") as sc_ps,
            tc.tile_pool(name="o_ps", bufs=1, space="PSUM") as o_ps,
            tc.tile_pool(name="rb_ps", bufs=2, space="PSUM") as rb_ps,
            tc.tile_pool(name="exps", bufs=4) as exps,
            tc.tile_pool(name="stage2", bufs=3) as stage2,
            tc.tile_pool(name="ostage", bufs=4) as ostage,
        ):
            for b in range(B):
                for qc in range(QC):
                    cols = slice(b * S + qc * 512, b * S + (qc + 1) * 512)
                    oom = o_ps.tile(
                        [HD + 1, HPC, 512], F32, tag="o", name="oom"
                    )
                    for kb in range(KB):
                        gkb = b * KB + kb
                        ks = slice(gkb * 128, (gkb + 1) * 128)
                        # issue the two heads' score matmuls back-to-back:
                        # K=64 each at partition bases 0/64 -> disjoint row
                        # groups, so the PE runs them concurrently
                        # both heads' scores into one 2-bank PSUM tile so
                        # a single 1024-wide exp covers them (halves the ACT
                        # per-op overhead and the PE's exp-wait points)
                        scp = sc_ps.tile(
                            [128, HPC, 512], F32, tag="s", name="scp"
                        )
                        for h in range(HPC):
                            hp = slice(h * HD, (h + 1) * HD)
                            nc.tensor.matmul(
                                scp[:, h, :],
                                lhsT=kT[hp, ks],
                                rhs=qT[hp, cols],
                                start=True,
                                stop=True,
                            )
                        ex = exps.tile(
                            [128, HPC, 512], BF16, tag="ex", name="ex"
                        )
                        nc.scalar.activation(
                            out=ex, in_=scp, func=AF.Exp, scale=SCALE
                        )
                        for h in range(HPC):
                            nc.tensor.matmul(
                                oom[:, h, :],
                                lhsT=vO[:, h, gkb, :],
                                rhs=ex[:, h, :],
                                start=(kb == 0),
                                stop=(kb == KB - 1),
                            )
                    # one wide ln over both heads' denominator rows,
                    # then per-head PE broadcast and exp(-x): 1/denom without
                    # the slow single-lane DVE reciprocal, no table switches
                    rc = stage2.tile([1, HPC, 512], F32R, tag="rc", name="rc")
                    nc.scalar.activation(
                        out=rc, in_=oom[HD : HD + 1, :, :], func=AF.Ln
                    )
                    for h in range(HPC):
                        rb = rb_ps.tile([HD, 512], F32, tag="rb", name="rb")
                        nc.tensor.matmul(
                            rb,
                            lhsT=ones_sb[0:1, 0:HD],
                            rhs=rc[:, h, :],
                            start=True,
                            stop=True,
                        )
                        rbs = stage2.tile([HD, 512], F32, tag=f"rbs{h}")
                        nc.scalar.activation(
                            out=rbs, in_=rb, func=AF.Exp, scale=-1.0
                        )
                        nc.vector.tensor_mul(
                            aT[h * HD : (h + 1) * HD, cols],
                            oom[0:HD, h, :],
                            rbs,
                        )
                    # fused partial projection for the 4 token blocks of
                    # this q-chunk (keeps the PE fed while the next chunk's
                    # exp tiles are produced; removes the separate proj tail)
                    for tbl in range(4):
                        tb = (b * QC + qc) * 4 + tbl
                        ob = ostage.tile([128, D], F32, tag="ob")
                        for nn in range(D // 512):
                            pps = rb_ps.tile(
                                [128, 512], F32, tag="rb", name="pps"
                            )
                            nc.tensor.matmul(
                                pps,
                                lhsT=aT[:, tb * 128 : (tb + 1) * 128],
                                rhs=wp_sb[:, nn * 512 : (nn + 1) * 512],
                                start=True,
                                stop=True,
                            )
                            # bias folded into the eviction (bias was pre-
                            # broadcast across partitions on the host)
                            nc.vector.tensor_add(
                                ob[:, nn * 512 : (nn + 1) * 512],
                                pps,
                                bpb_sb[:, nn * 512 : (nn + 1) * 512],
                            )
                        nc.sync.dma_start(
                            out=outp[tb * 128 : (tb + 1) * 128, :], in_=ob
                        )

    nc.compile()
    return nc


def make_in_maps(x, w_qkv, b_qkv, w_proj, b_proj, q_gamma, q_beta, k_gamma, k_beta,
                 affine):
    B, S, _ = x.shape
    T = B * S
    xT = np.ascontiguousarray(x.reshape(T, D).T)
    ones = np.ones((1, 512), np.float32)
    eye = np.eye(128, dtype=np.float32)
    in_maps = []
    for c in range(NCORES):
        rs = slice(c * DPC, (c + 1) * DPC)
        w_slice = np.concatenate(
            [w_qkv[rs], w_qkv[D:2 * D][rs.start:rs.stop], w_qkv[2 * D:][rs.start:rs.stop]],
            axis=0,
        )  # [384, 1024]
        b_slice = np.concatenate(
            [b_qkv[rs], b_qkv[D:2 * D][rs.start:rs.stop], b_qkv[2 * D:][rs.start:rs.stop]]
        )[None, :]  # [1, 384]
        import ml_dtypes
        bf = ml_dtypes.bfloat16
        m = {
            "xT": xT.astype(bf),
            "wt_qkv": np.ascontiguousarray(w_slice.T).astype(bf),
            "b_qkv_s": np.ascontiguousarray(b_slice).astype(bf),
            "wt_proj": np.ascontiguousarray(w_proj[:, rs].T).astype(bf),
            "c_bpb": np.ascontiguousarray(np.broadcast_to(
                b_proj[None, :] if c == 0 else np.zeros((1, D), np.float32),
                (128, D))).astype(np.float32),
            "c_ones": ones,
            "c_vones": np.ones((128, HPC, (T // 128), 1), bf),
            "c_onesb": np.ones((1, 512), bf),
            "c_eye": eye.astype(bf),
        }
        if affine:
            gb = np.stack([q_gamma, q_beta, k_gamma, k_beta])  # [4, 64]
            m["c_gb"] = np.ascontiguousarray(
                np.broadcast_to(gb[None], (128, 4, HD)).astype(np.float32)
            )
        in_maps.append(m)
    return in_maps


_NC_CACHE = {}

LAST_RESULTS = None


def kernel(x, w_qkv, b_qkv, w_proj, b_proj, q_gamma, q_beta, k_gamma, k_beta,
           **unused):
    global LAST_RESULTS
    x = np.asarray(x, np.float32)
    w_qkv = np.asarray(w_qkv, np.float32)
    b_qkv = np.asarray(b_qkv, np.float32)
    w_proj = np.asarray(w_proj, np.float32)
    b_proj = np.asarray(b_proj, np.float32)
    q_gamma = np.asarray(q_gamma, np.float32)
    q_beta = np.asarray(q_beta, np.float32)
    k_gamma = np.asarray(k_gamma, np.float32)
    k_beta = np.asarray(k_beta, np.float32)

    B, S, _ = x.shape
    affine = not (
        np.all(q_gamma == 1) and np.all(k_gamma == 1)
        and np.all(q_beta == 0) and np.all(k_beta == 0)
    )
    key = (B, S, affine)
    if key not in _NC_CACHE:
        _NC_CACHE[key] = build_nc(B, S, affine)
    nc = _NC_CACHE[key]

    in_maps = make_in_maps(
        x, w_qkv, b_qkv, w_proj, b_proj, q_gamma, q_beta, k_gamma, k_beta, affine
    )
    trace = bool(int(os.environ.get("BASS_KERNEL_TRACE", "0")))
    res = run_bass_kernel_spmd(
        nc, in_maps, core_ids=list(range(NCORES)), trace=trace
    )
    LAST_RESULTS = res
    acc = np.zeros((B * S, D), np.float32)
    for r in res.results:
        acc += r["outp"]
    return acc.reshape(B, S, D)
